# revision 1
# baseline (speedup 1.0000x reference)
"""ACT-LSTM (adaptive computation time) Bass/Tile kernel for 8 TRN2 NeuronCores.

Model (per batch row, up to 8 iterations):
    4 LSTM gates:  g = act(x @ Wx_g.T + bx_g + state @ Wh_g.T + bh_g)
    cell  = f*cell + i*c ; state = o*tanh(cell)
    out   = sigmoid(relu(state @ W1.T + b1) @ W2.T + b2)
    h     = sigmoid(state @ W_halt.T + b_halt); rows halt when cumsum(h) >= 1-eps
    final = sum_t out_t * halt_weight_t

Strategy:
  - Data-parallel: batch 8192 sharded to 8 cores (1024 rows each). Weights replicated.
  - Transposed layout everywhere: hidden dim on SBUF partitions, batch on the free
    dim, so the recurrent matmul needs no per-iteration transposes.
  - bf16 matmuls / storage, f32 accumulation + halting math (validated: rel err ~1e-4).
  - t=0 specialization: state==0, so gates need only the x-projection (K=64 matmuls).
  - Ragged-sequence exploitation: after t=1 the still-active rows are compacted
    (stream compaction via GPSIMD local_scatter with prefix-sum indices), so
    iteration 2 runs on <=512 columns instead of 1024. Iterations 3..7 are guarded
    by data-dependent Ifs and are skipped entirely once every row has halted.
  - Per-batch-row scalars (p_sum/act/acc/...) are packed on partitions {0,32,64,96}
    of three [128, BL] tiles (engine APs may only start at those partitions).
"""

import sys

sys.path.insert(0, "/opt/trn_rl_repo")

import numpy as np
import ml_dtypes

BATCH, IN, HID, OMID = 8192, 64, 1024, 128
MAX_ITER = 8
EPS = 1e-3
NCORES = 8
BL = BATCH // NCORES          # 1024 rows per core
NBLK = 512                    # matmul moving-dim (batch) block
NBLKS = BL // NBLK            # 2
PT = 128
KT = HID // PT                # 8 contraction tiles
JT = HID // PT                # 8 output-row tiles

_CACHE: dict = {}


def _build_nc(reps=1, upto='full', with_lib=True, opts=None):
    opts = dict(dict(gates_first=True, ft_hints=True), **(opts or {}))
    import concourse.mybir as mybir
    from concourse import bacc, library_config
    from concourse.tile import TileContext

    f32 = mybir.dt.float32
    bf16 = mybir.dt.bfloat16
    i16 = mybir.dt.int16
    i32 = mybir.dt.int32
    AF = mybir.ActivationFunctionType
    ALU = mybir.AluOpType

    nc = bacc.Bacc("TRN2", target_bir_lowering=False, debug=False,
                   enable_asserts=False)

    GATES = "ifco"
    # ---- DRAM parameters (all pre-transposed / pre-cast on host) ----
    d_xT = nc.dram_tensor("xT", [2 * IN, BL], bf16, kind="ExternalInput")
    d_WhT = {g: nc.dram_tensor(f"WhT{g}", [HID, HID], bf16, kind="ExternalInput")
             for g in GATES}
    # WxT packed two gates per tensor: i/f on partitions 0:64 / 64:128, c/o likewise
    d_WxT_if = nc.dram_tensor("WxTif", [2 * IN, HID], bf16, kind="ExternalInput")
    d_WxT_co = nc.dram_tensor("WxTco", [2 * IN, HID], bf16, kind="ExternalInput")
    d_bias = {g: nc.dram_tensor(f"bias{g}", [PT, JT], f32, kind="ExternalInput")
              for g in GATES}
    d_W1T = nc.dram_tensor("W1T", [HID, OMID], bf16, kind="ExternalInput")
    d_b1 = nc.dram_tensor("b1", [OMID, 1], f32, kind="ExternalInput")
    d_W2T = nc.dram_tensor("W2T", [OMID, 1], bf16, kind="ExternalInput")
    d_WhaltT = nc.dram_tensor("WhaltT", [PT, KT], bf16, kind="ExternalInput")
    d_sc = nc.dram_tensor("sc", [1, 2], f32, kind="ExternalInput")  # [b2, b_halt]
    d_iota1 = nc.dram_tensor("iota1", [16, BL], i16, kind="ExternalInput")
    d_ones = nc.dram_tensor("ones128", [1, PT], f32, kind="ExternalInput")
    d_out = nc.dram_tensor("out", [1, BL], f32, kind="ExternalOutput")

    with TileContext(nc) as tc:
        if with_lib:
            nc.gpsimd.load_library(library_config.local_scatter)

        with (
            tc.tile_pool(name="const", bufs=1) as cp,
            tc.tile_pool(name="work", bufs=3) as wp,
            tc.tile_pool(name="psg", bufs=2, space="PSUM") as pg,
        ):
            # ---------------- constants / weights -> SBUF ----------------
            def load(d, shape, dt_, tag):
                t = cp.tile(shape, dt_, tag=tag, name=tag)
                nc.sync.dma_start(out=t[:], in_=d.ap()[:, :])
                return t

            # DMA order matters: everything t=0 needs goes first, the big
            # recurrent weights (needed only from t=1) come last.
            xT_A = load(d_xT, [2 * IN, BL], bf16, "xT_A")
            WxTif = load(d_WxT_if, [2 * IN, HID], bf16, "WxTif")
            WxTco = load(d_WxT_co, [2 * IN, HID], bf16, "WxTco")
            WxT = {"i": WxTif[0:IN, :], "f": WxTif[IN:2 * IN, :],
                   "c": WxTco[0:IN, :], "o": WxTco[IN:2 * IN, :]}
            XOFF = {"i": 0, "c": 0, "f": IN, "o": IN}
            bias = {g: load(d_bias[g], [PT, JT], f32, f"bias{g}") for g in GATES}
            W1T = [cp.tile([PT, OMID], bf16, tag=f"W1T{kt}", name=f"W1T{kt}")
                   for kt in range(KT)]
            for kt in range(KT):
                nc.sync.dma_start(out=W1T[kt][:],
                                  in_=d_W1T.ap()[kt * PT:(kt + 1) * PT, :])
            b1 = load(d_b1, [OMID, 1], f32, "b1")
            W2T = load(d_W2T, [OMID, 1], bf16, "W2T")
            WhaltT = load(d_WhaltT, [PT, KT], bf16, "WhaltT")
            sc = load(d_sc, [1, 2], f32, "sc")
            iota1 = load(d_iota1, [16, BL], i16, "iota1")
            ones128 = load(d_ones, [1, PT], f32, "ones128")
            WhT = {g: [cp.tile([PT, HID], bf16, tag=f"WhT{g}{kt}",
                               name=f"WhT{g}{kt}") for kt in range(KT)]
                   for g in GATES}
            for g in GATES:
                for kt in range(KT):
                    nc.sync.dma_start(out=WhT[g][kt][:],
                                      in_=d_WhT[g].ap()[kt * PT:(kt + 1) * PT, :])

            stA = [cp.tile([PT, BL], bf16, tag=f"stA{kt}", name=f"stA{kt}")
                   for kt in range(KT)]
            clA = [cp.tile([PT, BL], bf16, tag=f"clA{kt}", name=f"clA{kt}")
                   for kt in range(KT)]
            # compaction daisy-chains into the (then dead) A tiles: the compact
            # buffers are [tmp, stA[0..6]]; xT compacts into stA[7]
            tmp_st = cp.tile([PT, BL], bf16, tag="tmp_st", name="tmp_st")
            tmp_cl = cp.tile([PT, BL], bf16, tag="tmp_cl", name="tmp_cl")
            stB = [tmp_st] + stA[:KT - 1]
            clB = [tmp_cl] + clA[:KT - 1]
            # dedicated compact-x destination so its scatter can run FIRST
            # (t2's first matmul chain starts from x)
            xT_B = cp.tile([2 * IN, BL], bf16, tag="xT_B", name="xT_B")

            # per-row scalars: separate [1, BL] tiles (engine tensor-tensor ops
            # require all SBUF operands at the same start partition)
            ROWS = {nm: cp.tile([1, BL], f32, tag=f"rv_{nm}", name=f"rv_{nm}")
                    for nm in ["act", "p", "acc", "ctr", "omp", "fin", "hw", "h",
                               "out"]}

            def row(nm, c0=0, c1=BL):
                return ROWS[nm][0:1, c0:c1]

            cb16 = cp.tile([16, BL], bf16, tag="cb16")    # contrib scatter staging
            sct16 = cp.tile([16, BL], bf16, tag="sct16")  # 16-channel scatter dst
            phi16 = cp.tile([16, BL], bf16, tag="phi16")  # p_sum hi part
            plo16 = cp.tile([16, BL], bf16, tag="plo16")  # p_sum lo part
            orig16 = cp.tile([16, BL], i16, tag="orig16")  # orig row ids (-1 invalid)
            # only row 0 of the 16-channel staging tiles carries data; the
            # scatters read all 16 rows, so initialize them once
            nc.vector.memset(cb16[:], 0.0)
            nc.vector.memset(phi16[:], 0.0)
            nc.vector.memset(plo16[:], 0.0)
            cum_row = cp.tile([1, BL], f32, tag="cum_row")
            dst_row = cp.tile([1, BL], f32, tag="dst_row")
            idx128 = cp.tile([PT, BL], i16, tag="idx128")

            # ---------------- shared pieces ----------------
            def mlp_and_halt(st, c0, w):
                """MLP head + halt logit for batch cols [c0, c0+w).
                Writes sigmoid outputs into rows out / h."""
                pm1 = pg.tile([OMID, NBLK], f32, tag="psgi", name="pm1")
                for kt in range(KT):
                    nc.tensor.matmul(pm1[:, :w], W1T[kt][:], st[kt][:, c0:c0 + w],
                                     start=(kt == 0), stop=(kt == KT - 1))
                relu1 = wp.tile([OMID, NBLK], bf16, tag="relu1", name="relu1")
                nc.scalar.activation(relu1[:, :w], pm1[:, :w], AF.Relu,
                                     bias=b1[:, 0:1], scale=1.0)
                ps2 = pg.tile([1, NBLK], f32, tag="psgf", name="ps2")
                nc.tensor.matmul(ps2[:, :w], W2T[:], relu1[:, :w],
                                 start=True, stop=True)
                psh = pg.tile([1, NBLK], f32, tag="psgc", name="psh")
                for kt in range(KT):
                    nc.tensor.matmul(psh[:, :w], WhaltT[:, kt:kt + 1],
                                     st[kt][:, c0:c0 + w],
                                     start=(kt == 0), stop=(kt == KT - 1))
                nc.scalar.activation(row("out", c0, c0 + w), ps2[:, :w], AF.Sigmoid,
                                     bias=sc[0:1, 0:1], scale=1.0)
                nc.scalar.activation(row("h", c0, c0 + w), psh[:, :w], AF.Sigmoid,
                                     bias=sc[0:1, 1:2], scale=1.0)

            def halt_math(c0, w, is_last):
                """Halting update on batch cols [c0, c0+w). Produces contrib row."""
                c1 = c0 + w
                p, a = row("p", c0, c1), row("act", c0, c1)
                omp, fin = row("omp", c0, c1), row("fin", c0, c1)
                hw_, h = row("hw", c0, c1), row("h", c0, c1)
                out, ctr = row("out", c0, c1), row("ctr", c0, c1)
                # omp = 1 - p_sum (old)
                nc.vector.tensor_scalar(out=omp, in0=p, scalar1=-1.0, scalar2=1.0,
                                        op0=ALU.mult, op1=ALU.add)
                # p_sum += h  (now p_new)
                nc.vector.tensor_add(out=p, in0=p, in1=h)
                if is_last:
                    nc.vector.tensor_copy(out=fin, in_=a)
                else:
                    nc.vector.tensor_scalar(out=fin, in0=p, scalar1=1.0 - EPS,
                                            scalar2=None, op0=ALU.is_ge)
                    nc.vector.tensor_mul(out=fin, in0=fin, in1=a)
                # hw = fin ? omp : h  ==  (omp - h)*fin + h   (then mask by act)
                nc.vector.tensor_sub(out=hw_, in0=omp, in1=h)
                nc.vector.tensor_mul(out=hw_, in0=hw_, in1=fin)
                nc.vector.tensor_add(out=hw_, in0=hw_, in1=h)
                nc.vector.tensor_mul(out=hw_, in0=hw_, in1=a)
                nc.vector.tensor_mul(out=ctr, in0=out, in1=hw_)
                nc.vector.tensor_sub(out=a, in0=a, in1=fin)

            def gates_block(t, st, cl, xT, c0, w):
                """Gate matmuls + cell/state update for cols [c0, c0+w).

                Two phases: (A) per jt: matmul chains, ACT evictions (no DVE
                dependency -> the strict-FIFO ACT queue never head-of-line
                blocks) and the DVE cell math; (B) tanh(cell) + state mul for
                all jt (by then every cell is ready)."""
                glist = "ico" if t == 0 else GATES
                o_tiles = []
                for jt in range(JT):
                    gt = {}
                    for g in glist:
                        psg = pg.tile([PT, NBLK], f32, tag=f"psg{g}",
                                      name=f"psg{g}")
                        nc.tensor.matmul(psg[:, :w],
                                         WxT[g][:, jt * PT:(jt + 1) * PT],
                                         xT[XOFF[g]:XOFF[g] + IN, c0:c0 + w],
                                         start=True, stop=(t == 0))
                        if t > 0:
                            for kt in range(KT):
                                nc.tensor.matmul(
                                    psg[:, :w],
                                    WhT[g][kt][:, jt * PT:(jt + 1) * PT],
                                    st[kt][:, c0:c0 + w],
                                    start=False, stop=(kt == KT - 1))
                        gtile = wp.tile([PT, NBLK], bf16, tag=f"g{g}",
                                        name=f"g{g}",
                                        bufs=(JT + 1 if g == "o" else None))
                        nc.scalar.activation(gtile[:, :w], psg[:, :w],
                                             AF.Tanh if g == "c" else AF.Sigmoid,
                                             bias=bias[g][:, jt:jt + 1],
                                             scale=1.0)
                        gt[g] = gtile
                    if t == 0:
                        # cell = i*c
                        nc.vector.tensor_mul(cl[jt][:, c0:c0 + w],
                                             gt["i"][:, :w], gt["c"][:, :w])
                    else:
                        # i *= c ; f *= cell ; cell = i + f  (in gate tiles)
                        nc.vector.tensor_mul(gt["i"][:, :w], gt["i"][:, :w],
                                             gt["c"][:, :w])
                        nc.vector.tensor_mul(gt["f"][:, :w], gt["f"][:, :w],
                                             cl[jt][:, c0:c0 + w])
                        nc.vector.tensor_add(cl[jt][:, c0:c0 + w],
                                             gt["i"][:, :w], gt["f"][:, :w])
                    o_tiles.append(gt["o"])
                for jt in range(JT):
                    # state = o * tanh(cell)
                    th = wp.tile([PT, NBLK], bf16, tag="th", name="th")
                    nc.scalar.activation(th[:, :w], cl[jt][:, c0:c0 + w],
                                         AF.Tanh)
                    nc.vector.tensor_mul(st[jt][:, c0:c0 + w],
                                         o_tiles[jt][:, :w], th[:, :w])

            def acc_add_direct(c0, w):
                c1 = c0 + w
                nc.vector.tensor_add(out=row("acc", c0, c1),
                                     in0=row("acc", c0, c1),
                                     in1=row("ctr", c0, c1))

            # ============ main body (repeatable for slope timing) ============
            REP = [0]

            def main_body():
              nc.vector.memset(row("p"), 0.0)
              nc.vector.memset(row("act"), 1.0)
              nc.vector.memset(row("acc"), 0.0)
              def dense_iter(t):
                  # all gate matmuls first so the PE stream never waits on the
                  # DVE state-update epilogue of the previous batch block
                  if opts["gates_first"]:
                      for nb in range(NBLKS):
                          gates_block(t, stA, clA, xT_A, nb * NBLK, NBLK)
                      for nb in range(NBLKS):
                          mlp_and_halt(stA, nb * NBLK, NBLK)
                          halt_math(nb * NBLK, NBLK, is_last=False)
                          acc_add_direct(nb * NBLK, NBLK)
                  else:
                      for nb in range(NBLKS):
                          gates_block(t, stA, clA, xT_A, nb * NBLK, NBLK)
                          mlp_and_halt(stA, nb * NBLK, NBLK)
                          halt_math(nb * NBLK, NBLK, is_last=False)
                          acc_add_direct(nb * NBLK, NBLK)

              # ------------- t = 0 (state==0: x-projection only) -------------
              dense_iter(0)

              # ---------------- t = 1 (dense) ----------------
              if upto == 't0':
                  return
              dense_iter(1)

              if upto == 't1':
                  return

              # total active count -> one register on every engine (fresh
              # tiles per call: the raw reg_load reads are not fully
              # WAR-tracked by Tile)
              def count_total(tagix):
                  cntf = cp.tile([1, 8], f32, tag=f"cntf{REP[0]}_{tagix}", name=f"cntf{REP[0]}_{tagix}")
                  cnti = cp.tile([1, 8], i32, tag=f"cnti{REP[0]}_{tagix}", name=f"cnti{REP[0]}_{tagix}")
                  nc.vector.reduce_sum(out=cntf[0:1, 0:1], in_=row("act"),
                                       axis=mybir.AxisListType.X)
                  nc.vector.tensor_copy(out=cnti[0:1, 0:1], in_=cntf[0:1, 0:1])
                  return nc.values_load(cnti[0:1, 0:1], min_val=0, max_val=BL,
                                        skip_runtime_bounds_check=True)

              def compaction_idx_half(nb):
                  # prefix-sum of act for one 512-col half (chained via the
                  # previous half's last element); dest = cumsum*act - 1.
                  # Half 0 only depends on t1 block 0's halting update, so it
                  # hides behind t1 block 1's matmuls.
                  c0, c1 = nb * NBLK, (nb + 1) * NBLK
                  nc.vector.tensor_tensor_scan(
                      out=cum_row[0:1, c0:c1], data0=row("act", c0, c1),
                      data1=row("act", c0, c1),
                      initial=0.0 if nb == 0 else cum_row[0:1, c0 - 1:c0],
                      op0=ALU.add, op1=ALU.max)
                  dst = dst_row[0:1, c0:c1]
                  nc.vector.tensor_mul(out=dst, in0=cum_row[0:1, c0:c1],
                                       in1=row("act", c0, c1))
                  nc.vector.tensor_scalar_add(out=dst, in0=dst, scalar1=-1.0)
                  # broadcast dest to 128 partitions (ones outer product; f32
                  # matmul is exact for these small integers)
                  ptag = "psgi" if nb == 0 else "psgf"
                  pb = pg.tile([PT, NBLK], f32, tag=ptag, name=f"pb{nb}")
                  nc.tensor.matmul(pb[:], ones128[:], dst, start=True, stop=True)
                  nc.vector.tensor_copy(out=idx128[:, c0:c1], in_=pb[:])

              def compaction():
                  # compact x first (t2's chains start from it), then state in
                  # chain order (dst k is src k+1), then cell
                  nc.gpsimd.local_scatter(xT_B[:], xT_A[:], idx128[:],
                                          2 * IN, BL, BL)
                  for kt in range(KT):
                      nc.gpsimd.local_scatter(stB[kt][:], stA[kt][:], idx128[:],
                                              PT, BL, BL)
                  for kt in range(KT):
                      nc.gpsimd.local_scatter(clB[kt][:], clA[kt][:], idx128[:],
                                              PT, BL, BL)
                  # compact p_sum as bf16 hi+lo split (exact to ~2^-17)
                  nc.vector.tensor_copy(out=phi16[0:1, :], in_=row("p"))
                  nc.vector.tensor_sub(out=plo16[0:1, :], in0=row("p"),
                                       in1=phi16[0:1, :])
                  nc.gpsimd.local_scatter(sct16[:], phi16[:], idx128[0:16, :],
                                          16, BL, BL)
                  nc.vector.tensor_copy(out=phi16[0:1, :], in_=sct16[0:1, :])
                  nc.gpsimd.local_scatter(sct16[:], plo16[:], idx128[0:16, :],
                                          16, BL, BL)
                  nc.vector.tensor_add(out=row("p"), in0=phi16[0:1, :],
                                       in1=sct16[0:1, :])
                  # compact original row ids (1-based -> 0-based; empty -> -1)
                  nc.gpsimd.local_scatter(orig16[:], iota1[:], idx128[0:16, :],
                                          16, BL, BL)
                  nc.vector.tensor_scalar(out=orig16[:], in0=orig16[:], scalar1=1,
                                          scalar2=None, op0=ALU.subtract)
                  # act := (slot occupied) in compact space
                  nc.vector.tensor_scalar(out=row("act"), in0=orig16[0:1, :],
                                          scalar1=0, scalar2=None, op0=ALU.is_ge)

              if upto == 'compact':
                  compaction_idx_half(0)
                  compaction_idx_half(1)
                  compaction()
                  return

              def block_work(t, c0, w):
                  gates_block(t, stB, clB, xT_B, c0, w)
                  mlp_and_halt(stB, c0, w)
                  halt_math(c0, w, is_last=(t == MAX_ITER - 1))
                  nc.vector.tensor_copy(out=cb16[0:1, c0:c0 + w],
                                        in_=row("ctr", c0, c0 + w))

              def acc_scatter():
                  # scatter contributions back to original row order
                  nc.gpsimd.local_scatter(sct16[:], cb16[:], orig16[:],
                                          16, BL, BL)
                  nc.vector.tensor_add(out=row("acc"), in0=row("acc"),
                                       in1=sct16[0:1, :])

              # timing variant: guard-free tail (correct only when every row
              # halts by t=2 in <=384 compact columns -- measurement only)
              if upto == 'noif':
                  compaction_idx_half(0)
                  compaction_idx_half(1)
                  compaction()
                  nc.vector.memset(cb16[0:1, :], 0.0)
                  block_work(2, 0, 384)
                  acc_scatter()
                  return

              # ---------------- t = 2..7 (compact space, fully guarded) -------
              def late_iter(t, cnt):
                  pf_skip = False if opts["ft_hints"] else None
                  with tc.If(cnt > 0, preferred_fallthrough_block=pf_skip):
                      nc.vector.memset(cb16[0:1, :], 0.0)
                      # actives may sit anywhere in compact space at t>=3 (no
                      # recompaction); run both halves under the one guard
                      block_work(t, 0, NBLK)
                      block_work(t, NBLK, NBLK)
                      acc_scatter()
                      if t < MAX_ITER - 1:
                          late_iter(t + 1, count_total(t))

              # t=2: compaction runs inside the guard (skipped if all rows
              # halted) so the scatters pipeline with t2's first matmul chains;
              # in compact space actives are contiguous, so block0 needs no
              # guard and later blocks guard on n2 thresholds.
              compaction_idx_half(0)
              compaction_idx_half(1)
              n2 = count_total(1)
              pf2 = True if opts["ft_hints"] else None
              pf_skip2 = False if opts["ft_hints"] else None
              # t2 block plan: compact actives are contiguous from col 0, so
              # use finer 384/384/256 blocks, each guarded by n2 thresholds
              with tc.If(n2 > 0, preferred_fallthrough_block=pf2):
                  compaction()
                  nc.vector.memset(cb16[0:1, :], 0.0)
                  block_work(2, 0, 384)
                  with tc.If(n2 > 384, preferred_fallthrough_block=pf_skip2):
                      block_work(2, 384, 384)
                  with tc.If(n2 > 768, preferred_fallthrough_block=pf_skip2):
                      block_work(2, 768, 256)
                  acc_scatter()
                  late_iter(3, count_total(2))

            for _rep in range(reps):
                REP[0] = _rep
                main_body()

            # ---------------- output ----------------
            nc.sync.dma_start(out=d_out.ap()[:, :], in_=row("acc"))

    nc.compile()
    return nc


def _prep_inputs(x, Wxi, bxi, Whi, bhi, Wxf, bxf, Whf, bhf, Wxc, bxc, Whc, bhc,
                 Wxo, bxo, Who, bho, W_halt, b_halt, W1, b1, W2, b2):
    bf = ml_dtypes.bfloat16
    gw = {"i": (Wxi, bxi, Whi, bhi), "f": (Wxf, bxf, Whf, bhf),
          "c": (Wxc, bxc, Whc, bhc), "o": (Wxo, bxo, Who, bho)}
    shared = {}
    for g, (Wx, bx, Wh, bh) in gw.items():
        shared[f"WhT{g}"] = np.ascontiguousarray(Wh.T).astype(bf)
        shared[f"bias{g}"] = np.ascontiguousarray(
            (bx + bh).astype(np.float32).reshape(JT, PT).T)
    shared["WxTif"] = np.ascontiguousarray(
        np.concatenate([gw["i"][0].T, gw["f"][0].T], axis=0)).astype(bf)
    shared["WxTco"] = np.ascontiguousarray(
        np.concatenate([gw["c"][0].T, gw["o"][0].T], axis=0)).astype(bf)
    shared["W1T"] = np.ascontiguousarray(W1.T).astype(bf)
    shared["b1"] = b1.astype(np.float32).reshape(OMID, 1)
    shared["W2T"] = np.ascontiguousarray(W2.T).astype(bf)
    shared["WhaltT"] = np.ascontiguousarray(
        W_halt.astype(np.float32).reshape(KT, PT).T).astype(bf)
    shared["sc"] = np.array([[b2[0], b_halt[0]]], dtype=np.float32)
    shared["iota1"] = np.tile(np.arange(1, BL + 1, dtype=np.int16), (16, 1))
    shared["ones128"] = np.ones((1, PT), dtype=np.float32)

    in_maps = []
    for c in range(NCORES):
        m = dict(shared)
        xs = x[c * BL:(c + 1) * BL].astype(np.float32)
        xt = np.ascontiguousarray(xs.T).astype(bf)
        m["xT"] = np.concatenate([xt, xt], axis=0)
        in_maps.append(m)
    return in_maps


def kernel(**inputs):
    from concourse.bass_utils import run_bass_kernel_spmd

    if "nc" not in _CACHE:
        _CACHE["nc"] = _build_nc()
    nc = _CACHE["nc"]

    in_maps = _prep_inputs(**{k: np.asarray(v) for k, v in inputs.items()})
    res = run_bass_kernel_spmd(nc, in_maps, core_ids=list(range(NCORES)))
    out = np.concatenate([res.results[c]["out"][0] for c in range(NCORES)])
    return out.reshape(BATCH, 1).astype(np.float32)

